# revision 5
# baseline (speedup 1.0000x reference)
"""TRN2 Bass kernel for nn_DAGLayer (gnn_message_passing).

DAG of 1x1 convs over [B=64, C=64, H=32, W=32]:
  preproc: s0 = W_pre[0] @ x0, s1 = W_pre[1] @ x1   (channel matmul)
  node i (i=0..3): s_{2+i} = sum_j conv1x1(relu(s_j), W_edge[...]) over all
  prior states j; output = concat(s2..s5) on channels -> [B, 256, H, W].

Strategy: data-parallel over batch across 8 NeuronCores (8 batches/core).
Every 1x1 conv is a channel-dim matmul over the N = H*W = 1024 spatial
columns of one batch. Matmul operands are fp16; accumulation is fp32 in
PSUM; raw node states go to HBM as fp16.

The kernel is elementwise-bound: every PSUM bank must be drained through
the Scalar(ACT)/Vector(DVE) PSUM read ports (1 elem/cycle/lane, DMA has
no PSUM route). Per batch there are 5 such streams (relu r01, relu r2,
cast [s2;s3], relu r4, cast [s4;s5]); r3 is derived from the fp16 SBUF
copy of s3 at DVE 4x rate. To amortize the fixed ~125-185 ns per-op
engine overheads, each stream is one FD=1024 op over a two-bank PSUM
tile (whole batch) instead of two FD=512 ops. PSUM budget: pP 2 banks +
pA 2 + pB 2x2 (double-buffered; the B-side dependency cycle
castB@k -> B1@k+1 is the longest) = 8 banks exactly. The A-side cycle is
cut by splitting castA per bank so A1a@k+1 can start after castA0@k.
"""
import sys

sys.path.insert(0, '/opt/trn_rl_repo')

import numpy as np

N_CORES = 8
B, C, H, W_SP = 64, 64, 32, 32
BP = B // N_CORES          # batches per core
HW = H * W_SP              # 1024 spatial columns per batch
NCOL = HW                  # one full batch per iteration

# Set by test harness to capture an NTFF trace; harmless default.
TRACE = False
LAST_RESULTS = None

_cache = {}


def _pack_weights(W_pre: np.ndarray, W_edge: np.ndarray) -> np.ndarray:
    """Pack all conv weights into one [128, 640] fp16 lhsT block.

    Layout (cols):
      0:128   WP  block-diag preproc: out [s0; s1] from rhs [x0; x1]
      128:256 A1  out [s2 | s3p] from rhs R01 = [r0; r1]
      256:384 B1  out [s4p | s5p] from rhs R01
      384:512 B2  out [s4p | s5p] from rhs R23 = [r2; r3]
      512:576 A2  (rows 0:64) edge r2->s3, written at PSUM partitions 64:128
      576:640 B3  (rows 0:64) edge r4->s5, written at PSUM partitions 64:128
    lhsT[k, m] = W[m, k] (pre-transposed for the PE's stationary operand).
    """
    Wt = np.zeros((128, 640), np.float32)
    T = lambda w: np.ascontiguousarray(w.T)
    Wt[0:64, 0:64] = T(W_pre[0])
    Wt[64:128, 64:128] = T(W_pre[1])
    # A1: cols 0:64 -> s2 (edges 0(r0), 1(r1)); cols 64:128 -> s3p (2, 3)
    Wt[0:64, 128:192] = T(W_edge[0])
    Wt[64:128, 128:192] = T(W_edge[1])
    Wt[0:64, 192:256] = T(W_edge[2])
    Wt[64:128, 192:256] = T(W_edge[3])
    # B1: cols 0:64 -> s4p (5(r0), 6(r1)); cols 64:128 -> s5p (9, 10)
    Wt[0:64, 256:320] = T(W_edge[5])
    Wt[64:128, 256:320] = T(W_edge[6])
    Wt[0:64, 320:384] = T(W_edge[9])
    Wt[64:128, 320:384] = T(W_edge[10])
    # B2 (rhs [r2; r3]): cols 0:64 -> s4p (7(r2), 8(r3)); cols 64:128 -> s5p (11, 12)
    Wt[0:64, 384:448] = T(W_edge[7])
    Wt[64:128, 384:448] = T(W_edge[8])
    Wt[0:64, 448:512] = T(W_edge[11])
    Wt[64:128, 448:512] = T(W_edge[12])
    # second-tier edges (K=64, weights at rows 0:64)
    Wt[0:64, 512:576] = T(W_edge[4])
    Wt[0:64, 576:640] = T(W_edge[13])
    return Wt.astype(np.float16)


def _build_program():
    import concourse.tile as tile
    from concourse import bacc, mybir

    F16, F32 = mybir.dt.float16, mybir.dt.float32
    Relu = mybir.ActivationFunctionType.Relu

    nc = bacc.Bacc()
    X = nc.dram_tensor("X", [BP, 128, HW], F16, kind="ExternalInput")
    Wt = nc.dram_tensor("Wt", [128, 640], F16, kind="ExternalInput")
    O = nc.dram_tensor("O", [BP, 256, HW], F16, kind="ExternalOutput")

    NIT = BP                        # one batch per iteration
    HN = NCOL // 2                  # 512: PSUM-bank half of a column tile
    with tile.TileContext(nc) as tc:
        with tc.tile_pool(name="wpool", bufs=1) as wpool, \
             tc.tile_pool(name="xpool", bufs=BP) as xpool, \
             tc.tile_pool(name="rpool", bufs=2) as rpool, \
             tc.tile_pool(name="opool", bufs=2) as opool, \
             tc.tile_pool(name="ppool", bufs=1, space="PSUM") as ppool, \
             tc.tile_pool(name="apool", bufs=1, space="PSUM") as apool, \
             tc.tile_pool(name="bpool", bufs=2, space="PSUM") as bpool:
            w = wpool.tile([128, 640], F16, tag="w")
            # weights on the Sync HWDGE ring; X loads on the GpSimd SWDGE
            # ring -- the two startup paths run in parallel.
            nc.sync.dma_start(w[:], Wt[:])
            xs = {}

            def load_x(it):
                if it >= NIT:
                    return
                xs[it] = xpool.tile([128, HW], F16, tag="x", name="x")
                # input loads ride the SWDGE queues (GpSimd is idle) so
                # they never queue behind output stores.
                nc.gpsimd.dma_start(xs[it][:], X[it])

            def warmup():
                # The PE HAM clock gate defaults to 1.2 GHz and only
                # releases to 2.4 GHz after ~3.4us of sustained matmul
                # activity. Burn that window on dummy matmuls over zeroed
                # SBUF while the startup DMAs are still in flight, so the
                # real matmuls run warm from iteration 0. Also fire a tiny
                # relu first so the ~1.3us ACT table load happens now, off
                # the critical path.
                dx = wpool.tile([128, HN], F16, tag="dx")
                dr = wpool.tile([128, 8], F16, tag="dr")
                nc.vector.memset(dx[:], 0.0)
                nc.scalar.activation(dr[:], dx[:, 0:8], Relu)
                dp = ppool.tile([128, HN], F32, tag="pP")
                for _ in range(12):
                    nc.tensor.matmul(dp[:], dx[:, 0:128], dx[:],
                                     start=True, stop=True)

            def stage_pre(it):
                # preproc: one K=128 block-diag matmul per PSUM bank half,
                # then one FD=1024 fused relu+cast on the Scalar engine.
                pP = ppool.tile([128, NCOL], F32, tag="pP")
                for h in (0, 1):
                    s = slice(h * HN, (h + 1) * HN)
                    nc.tensor.matmul(pP[:, s], w[:, 0:128], xs[it][:, s],
                                     start=True, stop=True)
                r01 = rpool.tile([128, NCOL], F16, tag="r01")
                nc.scalar.activation(r01[:], pP[:], Relu)
                return r01

            def stage_nodes(it, r01):
                # node pair A: pA = [s2; s3], node pair B: pB = [s4; s5]
                pA = apool.tile([128, NCOL], F32, tag="pA")
                pB = bpool.tile([128, NCOL], F32, tag="pB")
                for h in (0, 1):
                    s = slice(h * HN, (h + 1) * HN)
                    nc.tensor.matmul(pA[:, s], w[:, 128:256], r01[:, s],
                                     start=True, stop=False)
                # B1 only needs r01: keeps the PE busy while r2 runs.
                for h in (0, 1):
                    s = slice(h * HN, (h + 1) * HN)
                    nc.tensor.matmul(pB[:, s], w[:, 256:384], r01[:, s],
                                     start=True, stop=False)
                # r23 holds [r2; r3] as the K=128 rhs of the B2 matmuls.
                r23 = rpool.tile([128, NCOL], F16, tag="r23")
                nc.scalar.activation(r23[0:64, :], pA[0:64, :], Relu)     # r2
                # second-tier edge r2 -> s3 into PSUM partitions 64:128
                for h in (0, 1):
                    s = slice(h * HN, (h + 1) * HN)
                    nc.tensor.matmul(pA[64:128, s], w[0:64, 512:576],
                                     r23[0:64, s], start=False, stop=True,
                                     tile_position=(0, 64))
                # castA per bank so next iteration's A1 can reuse bank a0
                # as soon as the first half is drained (pA is
                # single-buffered).
                outA = opool.tile([128, NCOL], F16, tag="outA")
                for h in (0, 1):
                    s = slice(h * HN, (h + 1) * HN)
                    nc.vector.tensor_copy(outA[:, s], pA[:, s])
                # r3 from the fp16 SBUF copy on GpSimd: the only engine
                # with spare capacity (no PSUM involved, SBUF fp16 only).
                nc.gpsimd.tensor_relu(r23[64:128, :], outA[64:128, :])    # r3
                for h in (0, 1):
                    s = slice(h * HN, (h + 1) * HN)
                    nc.tensor.matmul(pB[:, s], w[:, 384:512], r23[:, s],
                                     start=False, stop=False)
                # r4 split 640/384 across both PSUM-capable engines so the
                # scalar and vector engines finish each iteration together.
                r4 = rpool.tile([64, NCOL], F16, tag="r4")
                R4S = 640
                nc.scalar.activation(r4[:, 0:R4S], pB[0:64, 0:R4S], Relu)
                nc.vector.tensor_relu(r4[:, R4S:NCOL], pB[0:64, R4S:NCOL])
                for h in (0, 1):
                    s = slice(h * HN, (h + 1) * HN)
                    nc.tensor.matmul(pB[64:128, s], w[0:64, 576:640],
                                     r4[:, s], start=False, stop=True,
                                     tile_position=(0, 64))
                outB = opool.tile([128, NCOL], F16, tag="outB")
                nc.vector.tensor_copy(outB[:], pB[:])

                # output channel order: s2 | s3 | s4 | s5
                nc.sync.dma_start(O[it, 0:128, :], outA[:])
                nc.sync.dma_start(O[it, 128:256, :], outB[:])

            # software pipeline: iteration it+1's preproc is emitted ahead
            # of iteration it's node stage so each in-order engine queue
            # always holds ready work. All X loads are issued upfront (16KB
            # of SBUF) and drain on the SWDGE ring in the background while
            # the PE warms up on dummy matmuls.
            warmup()
            for it in range(NIT):
                load_x(it)
            prev = stage_pre(0)
            for it in range(NIT):
                nxt = stage_pre(it + 1) if it + 1 < NIT else None
                stage_nodes(it, prev)
                prev = nxt
    nc.compile()
    return nc


def _get_program():
    if "nc" not in _cache:
        _cache["nc"] = _build_program()
    return _cache["nc"]


def kernel(x0, x1, W_pre, W_edge):
    global LAST_RESULTS
    from concourse.bass_utils import run_bass_kernel_spmd

    nc = _get_program()
    Xp = np.concatenate(
        [x0.reshape(B, C, HW), x1.reshape(B, C, HW)], axis=1)   # [B, 128, HW]
    Xp = Xp.astype(np.float16)
    Wt = _pack_weights(np.asarray(W_pre, np.float32), np.asarray(W_edge, np.float32))
    in_maps = [
        {"X": np.ascontiguousarray(Xp[i * BP:(i + 1) * BP]), "Wt": Wt}
        for i in range(N_CORES)
    ]
    res = run_bass_kernel_spmd(nc, in_maps, core_ids=list(range(N_CORES)),
                               trace=TRACE)
    LAST_RESULTS = res
    out = np.concatenate([r["O"] for r in res.results], axis=0).astype(np.float32)
    return np.ascontiguousarray(out.reshape(B, 4 * C, H, W_SP))


# revision 6
# speedup vs baseline: 2.8462x; 2.8462x over previous
"""TRN2 Bass kernel for nn_DAGLayer (gnn_message_passing).

DAG of 1x1 convs over [B=64, C=64, H=32, W=32]:
  preproc: s0 = W_pre[0] @ x0, s1 = W_pre[1] @ x1   (channel matmul)
  node i (i=0..3): s_{2+i} = sum_j conv1x1(relu(s_j), W_edge[...]) over all
  prior states j; output = concat(s2..s5) on channels -> [B, 256, H, W].

Strategy: data-parallel over batch across 8 NeuronCores (8 batches/core).
Every 1x1 conv is a channel-dim matmul over the N = H*W = 1024 spatial
columns of one batch. Matmul operands are fp16; accumulation is fp32 in
PSUM; raw node states go to HBM as fp16.

The kernel is elementwise-bound: every PSUM bank must be drained through
the Scalar(ACT)/Vector(DVE) PSUM read ports (1 elem/cycle/lane, DMA has
no PSUM route). Per batch there are 5 such streams (relu r01, relu r2,
cast [s2;s3], relu r4, cast [s4;s5]); r3 is derived from the fp16 SBUF
copy of s3 at DVE 4x rate. To amortize the fixed ~125-185 ns per-op
engine overheads, each stream is one FD=1024 op over a two-bank PSUM
tile (whole batch) instead of two FD=512 ops. PSUM budget: pP 2 banks +
pA 2 + pB 2x2 (double-buffered; the B-side dependency cycle
castB@k -> B1@k+1 is the longest) = 8 banks exactly. The A-side cycle is
cut by splitting castA per bank so A1a@k+1 can start after castA0@k.
"""
import sys

sys.path.insert(0, '/opt/trn_rl_repo')

import numpy as np

N_CORES = 8
B, C, H, W_SP = 64, 64, 32, 32
BP = B // N_CORES          # batches per core
HW = H * W_SP              # 1024 spatial columns per batch
NCOL = HW                  # one full batch per iteration

# Set by test harness to capture an NTFF trace; harmless default.
TRACE = False
LAST_RESULTS = None

_cache = {}


def _pack_weights(W_pre: np.ndarray, W_edge: np.ndarray) -> np.ndarray:
    """Pack all conv weights into one [128, 640] fp16 lhsT block.

    Layout (cols):
      0:128   WP  block-diag preproc: out [s0; s1] from rhs [x0; x1]
      128:256 A1  out [s2 | s3p] from rhs R01 = [r0; r1]
      256:384 B1  out [s4p | s5p] from rhs R01
      384:512 B2  out [s4p | s5p] from rhs R23 = [r2; r3]
      512:576 A2  (rows 0:64) edge r2->s3, written at PSUM partitions 64:128
      576:640 B3  (rows 0:64) edge r4->s5, written at PSUM partitions 64:128
    lhsT[k, m] = W[m, k] (pre-transposed for the PE's stationary operand).
    """
    Wt = np.zeros((128, 640), np.float32)
    T = lambda w: np.ascontiguousarray(w.T)
    Wt[0:64, 0:64] = T(W_pre[0])
    Wt[64:128, 64:128] = T(W_pre[1])
    # A1: cols 0:64 -> s2 (edges 0(r0), 1(r1)); cols 64:128 -> s3p (2, 3)
    Wt[0:64, 128:192] = T(W_edge[0])
    Wt[64:128, 128:192] = T(W_edge[1])
    Wt[0:64, 192:256] = T(W_edge[2])
    Wt[64:128, 192:256] = T(W_edge[3])
    # B1: cols 0:64 -> s4p (5(r0), 6(r1)); cols 64:128 -> s5p (9, 10)
    Wt[0:64, 256:320] = T(W_edge[5])
    Wt[64:128, 256:320] = T(W_edge[6])
    Wt[0:64, 320:384] = T(W_edge[9])
    Wt[64:128, 320:384] = T(W_edge[10])
    # B2 (rhs [r2; r3]): cols 0:64 -> s4p (7(r2), 8(r3)); cols 64:128 -> s5p (11, 12)
    Wt[0:64, 384:448] = T(W_edge[7])
    Wt[64:128, 384:448] = T(W_edge[8])
    Wt[0:64, 448:512] = T(W_edge[11])
    Wt[64:128, 448:512] = T(W_edge[12])
    # second-tier edges (K=64, weights at rows 0:64)
    Wt[0:64, 512:576] = T(W_edge[4])
    Wt[0:64, 576:640] = T(W_edge[13])
    return Wt.astype(np.float16)


def _build_program():
    import concourse.tile as tile
    from concourse import bacc, mybir

    F16, F32 = mybir.dt.float16, mybir.dt.float32
    Relu = mybir.ActivationFunctionType.Relu

    nc = bacc.Bacc()
    X = nc.dram_tensor("X", [BP, 128, HW], F16, kind="ExternalInput")
    Wt = nc.dram_tensor("Wt", [128, 640], F16, kind="ExternalInput")
    O = nc.dram_tensor("O", [BP, 256, HW], F16, kind="ExternalOutput")

    NIT = BP                        # one batch per iteration
    HN = NCOL // 2                  # 512: PSUM-bank half of a column tile
    with tile.TileContext(nc) as tc:
        with tc.tile_pool(name="wpool", bufs=1) as wpool, \
             tc.tile_pool(name="xpool", bufs=BP) as xpool, \
             tc.tile_pool(name="rpool", bufs=2) as rpool, \
             tc.tile_pool(name="opool", bufs=2) as opool, \
             tc.tile_pool(name="ppool", bufs=1, space="PSUM") as ppool, \
             tc.tile_pool(name="apool", bufs=1, space="PSUM") as apool, \
             tc.tile_pool(name="bpool", bufs=2, space="PSUM") as bpool:
            w = wpool.tile([128, 640], F16, tag="w")
            # weights on the Sync HWDGE ring; X loads on the GpSimd SWDGE
            # ring -- the two startup paths run in parallel.
            nc.sync.dma_start(w[:], Wt[:])
            xs = {}

            def load_x(it):
                if it >= NIT:
                    return
                xs[it] = xpool.tile([128, HW], F16, tag="x", name="x")
                # input loads ride the SWDGE queues (GpSimd is idle) so
                # they never queue behind output stores.
                nc.gpsimd.dma_start(xs[it][:], X[it])

            def warmup():
                # The PE HAM clock gate defaults to 1.2 GHz and only
                # releases to 2.4 GHz after ~3.4us of sustained matmul
                # activity. Burn that window on dummy matmuls over zeroed
                # SBUF while the startup DMAs are still in flight, so the
                # real matmuls run warm from iteration 0. Also fire a tiny
                # relu first so the ~1.3us ACT table load happens now, off
                # the critical path.
                dx = wpool.tile([128, HN], F16, tag="dx")
                dr = wpool.tile([128, 8], F16, tag="dr")
                nc.vector.memset(dx[:], 0.0)
                nc.scalar.activation(dr[:], dx[:, 0:8], Relu)
                dp = ppool.tile([128, HN], F32, tag="pP")
                for _ in range(12):
                    nc.tensor.matmul(dp[:], dx[:, 0:128], dx[:],
                                     start=True, stop=True)

            def stage_pre(it):
                # preproc: one K=128 block-diag matmul per PSUM bank half,
                # then one FD=1024 fused relu+cast on the Scalar engine.
                pP = ppool.tile([128, NCOL], F32, tag="pP")
                for h in (0, 1):
                    s = slice(h * HN, (h + 1) * HN)
                    nc.tensor.matmul(pP[:, s], w[:, 0:128], xs[it][:, s],
                                     start=True, stop=True)
                r01 = rpool.tile([128, NCOL], F16, tag="r01")
                nc.scalar.activation(r01[:], pP[:], Relu)
                return r01

            def stage_nodes(it, r01):
                # node pair A: pA = [s2; s3], node pair B: pB = [s4; s5]
                pA = apool.tile([128, NCOL], F32, tag="pA")
                pB = bpool.tile([128, NCOL], F32, tag="pB")
                for h in (0, 1):
                    s = slice(h * HN, (h + 1) * HN)
                    nc.tensor.matmul(pA[:, s], w[:, 128:256], r01[:, s],
                                     start=True, stop=False)
                # B1 only needs r01: keeps the PE busy while r2 runs.
                for h in (0, 1):
                    s = slice(h * HN, (h + 1) * HN)
                    nc.tensor.matmul(pB[:, s], w[:, 256:384], r01[:, s],
                                     start=True, stop=False)
                # r23 holds [r2; r3] as the K=128 rhs of the B2 matmuls.
                r23 = rpool.tile([128, NCOL], F16, tag="r23")
                nc.scalar.activation(r23[0:64, :], pA[0:64, :], Relu)     # r2
                # second-tier edge r2 -> s3 into PSUM partitions 64:128
                for h in (0, 1):
                    s = slice(h * HN, (h + 1) * HN)
                    nc.tensor.matmul(pA[64:128, s], w[0:64, 512:576],
                                     r23[0:64, s], start=False, stop=True,
                                     tile_position=(0, 64))
                # castA per bank so next iteration's A1 can reuse bank a0
                # as soon as the first half is drained (pA is
                # single-buffered).
                outA = opool.tile([128, NCOL], F16, tag="outA")
                for h in (0, 1):
                    s = slice(h * HN, (h + 1) * HN)
                    nc.vector.tensor_copy(outA[:, s], pA[:, s])
                # r3 from the fp16 SBUF copy: DVE 4x mode (~4 elem/cyc)
                # instead of a 1 elem/cyc PSUM read. (GpSimd's software
                # relu measures ~17us per op -- never route these there.)
                nc.vector.tensor_relu(r23[64:128, :], outA[64:128, :])    # r3
                for h in (0, 1):
                    s = slice(h * HN, (h + 1) * HN)
                    nc.tensor.matmul(pB[:, s], w[:, 384:512], r23[:, s],
                                     start=False, stop=False)
                r4 = rpool.tile([64, NCOL], F16, tag="r4")
                nc.scalar.activation(r4[:], pB[0:64, :], Relu)            # r4
                for h in (0, 1):
                    s = slice(h * HN, (h + 1) * HN)
                    nc.tensor.matmul(pB[64:128, s], w[0:64, 576:640],
                                     r4[:, s], start=False, stop=True,
                                     tile_position=(0, 64))
                outB = opool.tile([128, NCOL], F16, tag="outB")
                nc.vector.tensor_copy(outB[:], pB[:])

                # output channel order: s2 | s3 | s4 | s5
                nc.sync.dma_start(O[it, 0:128, :], outA[:])
                nc.sync.dma_start(O[it, 128:256, :], outB[:])

            # software pipeline: iteration it+1's preproc is emitted ahead
            # of iteration it's node stage so each in-order engine queue
            # always holds ready work. All X loads are issued upfront (16KB
            # of SBUF) and drain on the SWDGE ring in the background while
            # the PE warms up on dummy matmuls.
            warmup()
            for it in range(NIT):
                load_x(it)
            prev = stage_pre(0)
            for it in range(NIT):
                nxt = stage_pre(it + 1) if it + 1 < NIT else None
                stage_nodes(it, prev)
                prev = nxt
    nc.compile()
    return nc


def _get_program():
    if "nc" not in _cache:
        _cache["nc"] = _build_program()
    return _cache["nc"]


def kernel(x0, x1, W_pre, W_edge):
    global LAST_RESULTS
    from concourse.bass_utils import run_bass_kernel_spmd

    nc = _get_program()
    Xp = np.concatenate(
        [x0.reshape(B, C, HW), x1.reshape(B, C, HW)], axis=1)   # [B, 128, HW]
    Xp = Xp.astype(np.float16)
    Wt = _pack_weights(np.asarray(W_pre, np.float32), np.asarray(W_edge, np.float32))
    in_maps = [
        {"X": np.ascontiguousarray(Xp[i * BP:(i + 1) * BP]), "Wt": Wt}
        for i in range(N_CORES)
    ]
    res = run_bass_kernel_spmd(nc, in_maps, core_ids=list(range(N_CORES)),
                               trace=TRACE)
    LAST_RESULTS = res
    out = np.concatenate([r["O"] for r in res.results], axis=0).astype(np.float32)
    return np.ascontiguousarray(out.reshape(B, 4 * C, H, W_SP))


# revision 7
# speedup vs baseline: 4.0527x; 1.4239x over previous
"""TRN2 Bass kernel for nn_DAGLayer (gnn_message_passing).

DAG of 1x1 convs over [B=64, C=64, H=32, W=32]:
  preproc: s0 = W_pre[0] @ x0, s1 = W_pre[1] @ x1   (channel matmul)
  node i (i=0..3): s_{2+i} = sum_j conv1x1(relu(s_j), W_edge[...]) over all
  prior states j; output = concat(s2..s5) on channels -> [B, 256, H, W].

Strategy: data-parallel over batch across 8 NeuronCores (8 batches/core).
Every 1x1 conv is a channel-dim matmul over N = H*W spatial columns.
Matmul operands are fp16; accumulation is fp32 in PSUM; raw node states
are written out as fp16.

The kernel is elementwise-bound: every PSUM bank drains through the
Scalar/Vector PSUM read ports at 1 elem/cycle/lane (DMA has no PSUM
route), and per column-tile there are 5 such FD=512 streams (relu r01,
relu r2, relu r4, cast [s2;s3], cast [s4;s5]; r3 derives from the fp16
SBUF copy of s3 at DVE packed rate). Scalar takes the relus, Vector the
casts + r3 (~2.2us vs ~2.0us per tile).

To let the in-order engine queues run at their busy rate instead of the
~5.5us dependency chain, the emission is an explicit 4-stage modulo
schedule -- stage s of iteration t-s is emitted at step t, so each
engine's consecutive ops belong to different iterations and their
producers ran >= 1 period earlier:
  s0: pre-matmul + r01      s1: A1, r2, A2
  s2: castA, r3, B1, B2     s3: r4, B3, castB, stores
PSUM: ppool 2 + apool 3 + bpool 3 = 8 banks exactly.

Startup: the PE HAM clock gate defaults to half rate (1.2 GHz) until
~3.4us of sustained matmul activity, so a block of dummy matmuls over
zeroed SBUF runs while the input DMAs are in flight, and a tiny relu
preloads the ~1.3us ACT table. Weights ride the Sync HWDGE ring, inputs
the GpSimd SWDGE ring, in parallel.
"""
import sys

sys.path.insert(0, '/opt/trn_rl_repo')

import numpy as np

N_CORES = 8
B, C, H, W_SP = 64, 64, 32, 32
BP = B // N_CORES          # batches per core
HW = H * W_SP              # 1024 spatial columns per batch
NCOL = 512                 # matmul free-dim tile (one fp32 PSUM bank)

# Set by test harness to capture an NTFF trace; harmless default.
TRACE = False
LAST_RESULTS = None

_cache = {}


def _pack_weights(W_pre: np.ndarray, W_edge: np.ndarray) -> np.ndarray:
    """Pack all conv weights into one [128, 640] fp16 lhsT block.

    Layout (cols):
      0:128   WP  block-diag preproc: out [s0; s1] from rhs [x0; x1]
      128:256 A1  out [s2 | s3p] from rhs R01 = [r0; r1]
      256:384 B1  out [s4p | s5p] from rhs R01
      384:512 B2  out [s4p | s5p] from rhs R23 = [r2; r3]
      512:576 A2  (rows 0:64) edge r2->s3, written at PSUM partitions 64:128
      576:640 B3  (rows 0:64) edge r4->s5, written at PSUM partitions 64:128
    lhsT[k, m] = W[m, k] (pre-transposed for the PE's stationary operand).
    """
    Wt = np.zeros((128, 640), np.float32)
    T = lambda w: np.ascontiguousarray(w.T)
    Wt[0:64, 0:64] = T(W_pre[0])
    Wt[64:128, 64:128] = T(W_pre[1])
    # A1: cols 0:64 -> s2 (edges 0(r0), 1(r1)); cols 64:128 -> s3p (2, 3)
    Wt[0:64, 128:192] = T(W_edge[0])
    Wt[64:128, 128:192] = T(W_edge[1])
    Wt[0:64, 192:256] = T(W_edge[2])
    Wt[64:128, 192:256] = T(W_edge[3])
    # B1: cols 0:64 -> s4p (5(r0), 6(r1)); cols 64:128 -> s5p (9, 10)
    Wt[0:64, 256:320] = T(W_edge[5])
    Wt[64:128, 256:320] = T(W_edge[6])
    Wt[0:64, 320:384] = T(W_edge[9])
    Wt[64:128, 320:384] = T(W_edge[10])
    # B2 (rhs [r2; r3]): cols 0:64 -> s4p (7(r2), 8(r3)); cols 64:128 -> s5p (11, 12)
    Wt[0:64, 384:448] = T(W_edge[7])
    Wt[64:128, 384:448] = T(W_edge[8])
    Wt[0:64, 448:512] = T(W_edge[11])
    Wt[64:128, 448:512] = T(W_edge[12])
    # second-tier edges (K=64, weights at rows 0:64)
    Wt[0:64, 512:576] = T(W_edge[4])
    Wt[0:64, 576:640] = T(W_edge[13])
    return Wt.astype(np.float16)


def _build_program():
    import concourse.tile as tile
    from concourse import bacc, mybir

    F16, F32 = mybir.dt.float16, mybir.dt.float32
    Relu = mybir.ActivationFunctionType.Relu

    nc = bacc.Bacc()
    X = nc.dram_tensor("X", [BP, 128, HW], F16, kind="ExternalInput")
    Wt = nc.dram_tensor("Wt", [128, 640], F16, kind="ExternalInput")
    O = nc.dram_tensor("O", [BP, 256, HW], F16, kind="ExternalOutput")

    NIT = BP * HW // NCOL          # 16 column-tile iterations
    with tile.TileContext(nc) as tc:
        with tc.tile_pool(name="wpool", bufs=1) as wpool, \
             tc.tile_pool(name="xpool", bufs=BP) as xpool, \
             tc.tile_pool(name="rpool", bufs=3) as rpool, \
             tc.tile_pool(name="opool", bufs=3) as opool, \
             tc.tile_pool(name="ppool", bufs=2, space="PSUM") as ppool, \
             tc.tile_pool(name="apool", bufs=3, space="PSUM") as apool, \
             tc.tile_pool(name="bpool", bufs=3, space="PSUM") as bpool:
            w = wpool.tile([128, 640], F16, tag="w")
            # weights on the Sync HWDGE ring; X loads on the GpSimd SWDGE
            # ring -- the two startup paths run in parallel.
            nc.sync.dma_start(w[:], Wt[:])
            xs = {}
            for b in range(BP):
                xs[b] = xpool.tile([128, HW], F16, tag="x", name="x")
                nc.gpsimd.dma_start(xs[b][:], X[b])

            # PE warmup + ACT table preload over zeroed SBUF while the
            # input DMAs are still in flight.
            dx = wpool.tile([128, NCOL], F16, tag="dx")
            dr = wpool.tile([128, 8], F16, tag="dr")
            nc.vector.memset(dx[:], 0.0)
            nc.scalar.activation(dr[:], dx[:, 0:8], Relu)
            dp = ppool.tile([128, NCOL], F32, tag="pP")
            for _ in range(8):
                nc.tensor.matmul(dp[:], dx[:, 0:128], dx[:],
                                 start=True, stop=True)

            st = {}   # per-iteration live tiles

            def col(it):
                b, half = divmod(it, HW // NCOL)
                return b, slice(half * NCOL, (half + 1) * NCOL)

            def stage0(it):
                b, s = col(it)
                pP = ppool.tile([128, NCOL], F32, tag="pP")
                nc.tensor.matmul(pP[:], w[:, 0:128], xs[b][:, s],
                                 start=True, stop=True)
                r01 = rpool.tile([128, NCOL], F16, tag="r01")
                nc.scalar.activation(r01[:], pP[:], Relu)
                st[it] = {"r01": r01}

            def stage1(it):
                d = st[it]
                r01 = d["r01"]
                pA = apool.tile([128, NCOL], F32, tag="pA")
                nc.tensor.matmul(pA[:], w[:, 128:256], r01[:],
                                 start=True, stop=False)
                r23 = rpool.tile([128, NCOL], F16, tag="r23")
                nc.scalar.activation(r23[0:64, :], pA[0:64, :], Relu)     # r2
                nc.tensor.matmul(pA[64:128, :], w[0:64, 512:576],
                                 r23[0:64, :], start=False, stop=True,
                                 tile_position=(0, 64))
                d["pA"], d["r23"] = pA, r23

            def stage2(it):
                d = st[it]
                r01, pA, r23 = d["r01"], d["pA"], d["r23"]
                outA = opool.tile([128, NCOL], F16, tag="outA")
                nc.vector.tensor_copy(outA[:], pA[:])
                # r3 from the fp16 SBUF copy: DVE packed mode instead of a
                # 1 elem/cyc PSUM read.
                nc.vector.tensor_relu(r23[64:128, :], outA[64:128, :])    # r3
                pB = bpool.tile([128, NCOL], F32, tag="pB")
                nc.tensor.matmul(pB[:], w[:, 256:384], r01[:],
                                 start=True, stop=False)
                nc.tensor.matmul(pB[:], w[:, 384:512], r23[:],
                                 start=False, stop=False)
                d["pB"], d["outA"] = pB, outA

            def stage3(it):
                d = st.pop(it)
                pB, outA = d["pB"], d["outA"]
                b, s = col(it)
                r4 = rpool.tile([64, NCOL], F16, tag="r4")
                nc.scalar.activation(r4[:], pB[0:64, :], Relu)            # r4
                nc.tensor.matmul(pB[64:128, :], w[0:64, 576:640], r4[:],
                                 start=False, stop=True,
                                 tile_position=(0, 64))
                outB = opool.tile([128, NCOL], F16, tag="outB")
                nc.vector.tensor_copy(outB[:], pB[:])
                # output channel order: s2 | s3 | s4 | s5
                nc.sync.dma_start(O[b, 0:128, s], outA[:])
                nc.sync.dma_start(O[b, 128:256, s], outB[:])

            stages = [stage0, stage1, stage2, stage3]
            S = len(stages)
            for t in range(NIT + S - 1):
                for si in range(S):
                    it = t - si
                    if 0 <= it < NIT:
                        stages[si](it)
    nc.compile()
    return nc


def _get_program():
    if "nc" not in _cache:
        _cache["nc"] = _build_program()
    return _cache["nc"]


def kernel(x0, x1, W_pre, W_edge):
    global LAST_RESULTS
    from concourse.bass_utils import run_bass_kernel_spmd

    nc = _get_program()
    Xp = np.concatenate(
        [x0.reshape(B, C, HW), x1.reshape(B, C, HW)], axis=1)   # [B, 128, HW]
    Xp = Xp.astype(np.float16)
    Wt = _pack_weights(np.asarray(W_pre, np.float32), np.asarray(W_edge, np.float32))
    in_maps = [
        {"X": np.ascontiguousarray(Xp[i * BP:(i + 1) * BP]), "Wt": Wt}
        for i in range(N_CORES)
    ]
    res = run_bass_kernel_spmd(nc, in_maps, core_ids=list(range(N_CORES)),
                               trace=TRACE)
    LAST_RESULTS = res
    out = np.concatenate([r["O"] for r in res.results], axis=0).astype(np.float32)
    return np.ascontiguousarray(out.reshape(B, 4 * C, H, W_SP))
